# revision 1
# baseline (speedup 1.0000x reference)
"""Dense MLP y = x @ W.T + b on 8 TRN2 NeuronCores, data-parallel over batch.

Full inputs: x [8192, 1024] f32, W [1024, 1024] f32, b [1024] f32.
Each core computes a [1024, 1024] slice of the output.

Per-core kernel computes the transposed output
    outT[n, m] = sum_k WT[k, n] * xT[k, m] + b[n]
so the bias lands on the partition dim (n) and fuses into the PSUM
eviction as a DVE tensor_scalar add. Host pre-transposes x-shards and W
to K-major (contraction on partitions) and un-transposes the gathered
outputs; only device time counts.

Raw Bass (no TileContext: its exit drain trips "Too many sync wait
commands" in this compiler build).

Engine layout (v3, trace-driven):
  sync:   ALL load DMAs on one HWDGE queue, in exact first-use order
          ([wt_c0[k], xt_c0[k]] pairs, then wt_c1, then xt_c1). The
          aggregate DMA rate caps at ~400 GB/s no matter how many
          queues issue, and queues do NOT share bandwidth fairly, so
          one priority-ordered queue beats two racing ones.
  scalar: output stores (idle queue; stores only need a ~130 GB/s
          trickle and must not displace load descriptors).
  gpsimd: bias load (SWDGE, off the critical queues).
  tensor: four k-outer phases over 4 PSUM banks each with per-slice
          gating - each k-slice feeds 4 matmuls the moment it lands,
          so only ~7 us of compute remains after the last load byte.
  vector: PSUM->SBUF evictions with fused bias add.
All matmul operands are float32r end to end (DRAM + SBUF) - the BIR
verifier requires fp32r matmul inputs to be produced as fp32r, and
fp32r streams 4x faster than plain fp32 through the PE at moving dim
512 (1 cycle/row).
"""

import numpy as np

import concourse.bass as bass
import concourse.mybir as mybir
from concourse.bass_utils import run_bass_kernel_spmd

B, IN_F, OUT_F = 8192, 1024, 1024
N_CORES = 8
M = B // N_CORES  # batch rows per core
P = 128           # partitions
MB = 512          # moving-dim block (one PSUM bank of fp32)
KT = IN_F // P    # k tiles (8)
NT = OUT_F // P   # n tiles (8)
CB = 512          # column-block width (2KB DMA lines per partition)
NGROUPS = (M // MB) * NT  # 16 psum groups, order g = mb*NT + nt

F32 = mybir.dt.float32
F32R = mybir.dt.float32r


def build_program() -> bass.Bass:
    nc = bass.Bass()
    xT = nc.declare_dram_parameter("xT", [IN_F, M], F32R, isOutput=False)
    wT = nc.declare_dram_parameter("wT", [IN_F, OUT_F], F32R, isOutput=False)
    bias = nc.declare_dram_parameter("bias", [P, NT], F32, isOutput=False)
    outT = nc.declare_dram_parameter("outT", [OUT_F, M], F32, isOutput=True)

    import contextlib

    with contextlib.ExitStack() as ctx:
        wt_sb = [
            [ctx.enter_context(nc.sbuf_tensor(f"wt{k}_{c}", [P, CB], F32R))
             for c in range(2)]
            for k in range(KT)
        ]
        xt_sb = [
            [ctx.enter_context(nc.sbuf_tensor(f"xt{k}_{c}", [P, CB], F32R))
             for c in range(2)]
            for k in range(KT)
        ]
        ot_sb = [
            ctx.enter_context(nc.sbuf_tensor(f"ot{j}", [P, MB], F32))
            for j in range(4)
        ]
        bias_sb = ctx.enter_context(nc.sbuf_tensor("bias_sb", [P, NT], F32))
        ps = [
            ctx.enter_context(nc.psum_tensor(f"ps{b}", [P, MB], F32))
            for b in range(8)
        ]
        ld_b = ctx.enter_context(nc.semaphore("ld_b"))
        # Per (k-slice, column-block) load sems: a shared counter can't
        # prove a *specific* DMA finished (completions are unordered),
        # a single-incrementer sem can.
        ld_w = [
            [ctx.enter_context(nc.semaphore(f"ld_w{k}_{c}")) for c in range(2)]
            for k in range(KT)
        ]
        ld_x = [
            [ctx.enter_context(nc.semaphore(f"ld_x{k}_{c}")) for c in range(2)]
            for k in range(KT)
        ]
        mm = ctx.enter_context(nc.semaphore("mm"))
        ev = ctx.enter_context(nc.semaphore("ev"))
        ev_h = ctx.enter_context(nc.semaphore("ev_h"))  # last-group halves
        # Per-ot-slot store sems (same unordered-completion argument).
        st_sems = [
            ctx.enter_context(nc.semaphore(f"st{j}")) for j in range(4)
        ]

        with nc.Block(no_gpsimd_drain=True) as block:

            @block.sync
            def _(sync):
                # ALL loads on one FIFO queue in exact first-use order:
                # the DMA fabric caps at ~390-400 GB/s aggregate no
                # matter how many queues issue (two queues just split it
                # and scramble the priority order).
                for k in range(KT):
                    sync.dma_start(
                        out=wt_sb[k][0][:],
                        in_=wT[k * P:(k + 1) * P, 0:CB],
                    ).then_inc(ld_w[k][0], 16)
                    sync.dma_start(
                        out=xt_sb[k][0][:],
                        in_=xT[k * P:(k + 1) * P, 0:CB],
                    ).then_inc(ld_x[k][0], 16)
                for k in range(KT):
                    sync.dma_start(
                        out=wt_sb[k][1][:],
                        in_=wT[k * P:(k + 1) * P, CB:2 * CB],
                    ).then_inc(ld_w[k][1], 16)
                for k in range(KT):
                    sync.dma_start(
                        out=xt_sb[k][1][:],
                        in_=xT[k * P:(k + 1) * P, CB:2 * CB],
                    ).then_inc(ld_x[k][1], 16)

            @block.gpsimd
            def _(gpsimd):
                gpsimd.dma_start(out=bias_sb[:], in_=bias[:]).then_inc(ld_b, 16)

            @block.scalar
            def _(scalar):
                # Stores on the idle scalar queue (~130 GB/s trickle,
                # must not displace load descriptors). Last group is
                # split in half so the final eviction->store->drain
                # chain is shorter.
                for g in range(NGROUPS - 1):
                    mb, nt = divmod(g, NT)
                    scalar.wait_ge(ev, g + 1)
                    scalar.dma_start(
                        out=outT[nt * P:(nt + 1) * P, mb * MB:(mb + 1) * MB],
                        in_=ot_sb[g % 4][:],
                    ).then_inc(st_sems[g % 4], 16)
                for h in range(2):
                    scalar.wait_ge(ev_h, h + 1)
                    scalar.dma_start(
                        out=outT[7 * P:8 * P,
                                 MB + h * (MB // 2):MB + (h + 1) * (MB // 2)],
                        in_=ot_sb[3][:, h * (MB // 2):(h + 1) * (MB // 2)],
                    ).then_inc(st_sems[3], 16)
                for j in range(3):
                    scalar.wait_ge(st_sems[j], (NGROUPS // 4) * 16)
                scalar.wait_ge(st_sems[3], 5 * 16)

            @block.tensor
            def _(tensor):
                # Three k-outer phases over 4 PSUM banks each. Group ids
                # (= mm/ev order): P0 -> g0-3 (nt0-3, mb0, banks 0-3),
                # P1 -> g4-7 (nt4-7, mb0, banks 4-7), P2 -> g8-11
                # (nt0-3, mb1, banks 0-3). Each k-slice feeds 4 matmuls
                # as soon as it lands.
                for phase in range(3):
                    mb = phase // 2          # 0,0,1
                    cw = phase % 2           # wt column block 0,1,0
                    bank0 = cw * 4           # banks 0-3 / 4-7
                    if phase == 2:
                        tensor.wait_ge(ev, 4)   # banks 0-3 evicted (P0)
                    for k in range(KT):
                        if phase == 0:
                            tensor.wait_ge(ld_w[k][0], 16)
                            tensor.wait_ge(ld_x[k][0], 16)
                        elif phase == 1:
                            tensor.wait_ge(ld_w[k][1], 16)
                        elif phase == 2:
                            tensor.wait_ge(ld_x[k][1], 16)
                        for j in range(4):
                            inst = tensor.matmul(
                                ps[bank0 + j][:, :],
                                wt_sb[k][cw][:, j * P:(j + 1) * P],
                                xt_sb[k][mb][:, :],
                                start=(k == 0),
                                stop=(k == KT - 1),
                            )
                            if k == KT - 1:
                                inst.then_inc(mm, 1)
                # Last phase (nt4-7, mb1, banks 4-7) k-inner: group
                # completions land ~1.9us apart so evictions + stores
                # pipeline instead of bunching at the end.
                tensor.wait_ge(ev, 8)   # banks 4-7 evicted (P1)
                for g in range(12, NGROUPS):
                    nt = g - 8
                    ni = nt - 4
                    inst = None
                    for k in range(KT):
                        inst = tensor.matmul(
                            ps[4 + ni][:, :],
                            wt_sb[k][1][:, ni * P:(ni + 1) * P],
                            xt_sb[k][1][:, :],
                            start=(k == 0),
                            stop=(k == KT - 1),
                        )
                    inst.then_inc(mm, 1)

            @block.vector
            def _(vector):
                vector.wait_ge(ld_b, 16)
                for g in range(NGROUPS - 1):
                    mb, nt = divmod(g, NT)
                    vector.wait_ge(mm, g + 1)
                    if g >= 4:
                        # ot slot g%4 reused: all issued slot stores
                        # (groups g%4, g%4+4, ..., g-4) must be done
                        vector.wait_ge(st_sems[g % 4], (g // 4) * 16)
                    vector.tensor_scalar_add(
                        ot_sb[g % 4][:],
                        ps[g % 8][:, :],
                        bias_sb[:, nt:nt + 1],
                    ).then_inc(ev, 1)
                # Last group in halves: first half's store overlaps the
                # second half's eviction, shortening the critical tail.
                vector.wait_ge(mm, NGROUPS)
                vector.wait_ge(st_sems[3], 48)
                for h in range(2):
                    vector.tensor_scalar_add(
                        ot_sb[3][:, h * (MB // 2):(h + 1) * (MB // 2)],
                        ps[7][:, h * (MB // 2):(h + 1) * (MB // 2)],
                        bias_sb[:, 7:8],
                    ).then_inc(ev_h, 1)

    return nc


_PROGRAM = None


def _get_program() -> bass.Bass:
    global _PROGRAM
    if _PROGRAM is None:
        _PROGRAM = build_program()
    return _PROGRAM


def make_in_maps(x: np.ndarray, W: np.ndarray, b: np.ndarray) -> list[dict]:
    WT = np.ascontiguousarray(W.T.astype(np.float32, copy=False))
    bias = np.ascontiguousarray(
        b.astype(np.float32, copy=False).reshape(NT, P).T
    )
    in_maps = []
    for c in range(N_CORES):
        xT = np.ascontiguousarray(
            x[c * M:(c + 1) * M, :].T.astype(np.float32, copy=False)
        )
        in_maps.append({"xT": xT, "wT": WT, "bias": bias})
    return in_maps


def assemble_output(results: list[dict]) -> np.ndarray:
    out = np.empty((B, OUT_F), dtype=np.float32)
    for c in range(N_CORES):
        out[c * M:(c + 1) * M, :] = results[c]["outT"].T
    return out


def kernel(x: np.ndarray, W: np.ndarray, b: np.ndarray) -> np.ndarray:
    nc = _get_program()
    in_maps = make_in_maps(np.asarray(x), np.asarray(W), np.asarray(b))
    res = run_bass_kernel_spmd(nc, in_maps, list(range(N_CORES)))
    return assemble_output(res.results)



# revision 4
# speedup vs baseline: 1.1892x; 1.1892x over previous
"""Dense MLP y = x @ W.T + b on 8 TRN2 NeuronCores, data-parallel over batch.

Full inputs: x [8192, 1024] f32, W [1024, 1024] f32, b [1024] f32.
Each core computes a [1024, 1024] slice of the output.

Per-core kernel computes the transposed output
    outT[n, m] = sum_k WT[k, n] * xT[k, m] + b[n]
so the bias lands on the partition dim (n) and fuses into the PSUM
eviction as a DVE tensor_scalar add. Host pre-transposes x-shards and W
to K-major (contraction on partitions) and un-transposes the gathered
outputs; only device time counts.

v4 (trace-driven, fp16): the warm PE streams 1 row/cycle for fp32r,
fp16 and bf16 alike (227 ns per 512-row matmul measured), so the
128-matmul PE floor is ~29 us and fp32 DMA (8 MB loads at ~270 GB/s on
one HWDGE ring) was the binding constraint. Switch x/W/out to fp16
(max-rel-err ~7e-4, gate is 2e-2):
  - loads drop to 4 MB, split across BOTH HWDGE rings (sync ring: W,
    scalar ring: x) which round-robin fairly at packet granularity, so
    w_k/x_k tiles land in lockstep well ahead of the PE;
  - stores drop to 2 MB and alternate rings by group parity.
Tensor program: ~0.9 us of tiny dummy matmuls at t=0 (on a memset
tile) trip the HAM activity window early so the PE reaches 2.4 GHz
sooner; phase A (mb=0) runs k-outer so each arriving k-slice feeds 8
matmuls; phase B (mb=1) runs k-inner per group so group completions
pace 1.8 us apart and evictions+stores pipeline instead of bunching.
The last group is evicted/stored in halves on both rings to shorten
the tail. Raw Bass (no TileContext: its exit drain trips "Too many
sync wait commands" in this compiler build).
"""

import numpy as np

import concourse.bass as bass
import concourse.mybir as mybir
from concourse.bass_utils import run_bass_kernel_spmd

B, IN_F, OUT_F = 8192, 1024, 1024
N_CORES = 8
M = B // N_CORES  # batch rows per core (1024)
P = 128           # partitions
MB = 512          # moving-dim block (one PSUM bank of fp32)
KT = IN_F // P    # k tiles (8)
NT = OUT_F // P   # n tiles (8)
NGROUPS = 16      # (mb, nt) output groups of [128, 512]

F16 = mybir.dt.float16
F32 = mybir.dt.float32


def build_program() -> bass.Bass:
    nc = bass.Bass()
    xT = nc.declare_dram_parameter("xT", [IN_F, M], F16, isOutput=False)
    wT = nc.declare_dram_parameter("wT", [IN_F, OUT_F], F16, isOutput=False)
    bias = nc.declare_dram_parameter("bias", [P, NT], F32, isOutput=False)
    outT = nc.declare_dram_parameter("outT", [OUT_F, M], F16, isOutput=True)

    import contextlib

    with contextlib.ExitStack() as ctx:
        wt_sb = [
            ctx.enter_context(nc.sbuf_tensor(f"wt{k}", [P, OUT_F], F16))
            for k in range(KT)
        ]
        xt_sb = [
            ctx.enter_context(nc.sbuf_tensor(f"xt{k}", [P, M], F16))
            for k in range(KT)
        ]
        ot_sb = [
            ctx.enter_context(nc.sbuf_tensor(f"ot{j}", [P, MB], F16))
            for j in range(8)
        ]
        bias_sb = ctx.enter_context(nc.sbuf_tensor("bias_sb", [P, NT], F32))
        dummy_sb = ctx.enter_context(nc.sbuf_tensor("dummy_sb", [P, P], F16))
        ps = [
            ctx.enter_context(nc.psum_tensor(f"ps{b}", [P, MB], F32))
            for b in range(8)
        ]
        ld_b = ctx.enter_context(nc.semaphore("ld_b"))
        dm = ctx.enter_context(nc.semaphore("dm"))
        # Per-tile load sems: a shared counter can't prove a *specific*
        # DMA finished (completions are unordered), a single-incrementer
        # sem can. k=0 tiles are split in column halves for a fast PE
        # start.
        ld_w0 = [ctx.enter_context(nc.semaphore(f"ld_w0{c}")) for c in range(2)]
        ld_x0 = [ctx.enter_context(nc.semaphore(f"ld_x0{c}")) for c in range(2)]
        ld_w = [ctx.enter_context(nc.semaphore(f"ld_w{k}")) for k in range(1, KT)]
        ld_x = [ctx.enter_context(nc.semaphore(f"ld_x{k}")) for k in range(1, KT)]
        mm = ctx.enter_context(nc.semaphore("mm"))
        ev = ctx.enter_context(nc.semaphore("ev"))
        ev_h = ctx.enter_context(nc.semaphore("ev_h"))  # last-group halves
        st_sems = [ctx.enter_context(nc.semaphore(f"st{j}")) for j in range(8)]
        st_h = ctx.enter_context(nc.semaphore("st_h"))

        def store_ap(g):
            mb, nt = divmod(g, NT)
            return outT[nt * P:(nt + 1) * P, mb * MB:(mb + 1) * MB]

        with nc.Block(no_gpsimd_drain=True) as block:

            @block.sync
            def _(sync):
                # W loads on the sync HWDGE ring, in k order.
                sync.dma_start(
                    out=wt_sb[0][:, 0:MB], in_=wT[0:P, 0:MB],
                ).then_inc(ld_w0[0], 16)
                sync.dma_start(
                    out=wt_sb[0][:, MB:2 * MB], in_=wT[0:P, MB:2 * MB],
                ).then_inc(ld_w0[1], 16)
                for k in range(1, KT):
                    sync.dma_start(
                        out=wt_sb[k][:], in_=wT[k * P:(k + 1) * P, :],
                    ).then_inc(ld_w[k - 1], 16)
                # Even-group stores on this ring.
                for g in range(0, NGROUPS - 1, 2):
                    sync.wait_ge(ev, g + 1)
                    sync.dma_start(
                        out=store_ap(g), in_=ot_sb[g % 8][:],
                    ).then_inc(st_sems[g % 8], 16)
                # First half of the final group.
                sync.wait_ge(ev_h, 1)
                sync.dma_start(
                    out=outT[7 * P:8 * P, MB:MB + MB // 2],
                    in_=ot_sb[7][:, 0:MB // 2],
                ).then_inc(st_h, 16)
                for j in range(0, 8, 2):
                    sync.wait_ge(st_sems[j], 32)
                sync.wait_ge(st_h, 32)

            @block.scalar
            def _(scalar):
                # x loads on the scalar HWDGE ring, in k order.
                scalar.dma_start(
                    out=xt_sb[0][:, 0:MB], in_=xT[0:P, 0:MB],
                ).then_inc(ld_x0[0], 16)
                scalar.dma_start(
                    out=xt_sb[0][:, MB:2 * MB], in_=xT[0:P, MB:2 * MB],
                ).then_inc(ld_x0[1], 16)
                for k in range(1, KT):
                    scalar.dma_start(
                        out=xt_sb[k][:], in_=xT[k * P:(k + 1) * P, :],
                    ).then_inc(ld_x[k - 1], 16)
                # Odd-group stores on this ring.
                for g in range(1, NGROUPS - 1, 2):
                    scalar.wait_ge(ev, g + 1)
                    scalar.dma_start(
                        out=store_ap(g), in_=ot_sb[g % 8][:],
                    ).then_inc(st_sems[g % 8], 16)
                # Second half of the final group.
                scalar.wait_ge(ev_h, 2)
                scalar.dma_start(
                    out=outT[7 * P:8 * P, MB + MB // 2:2 * MB],
                    in_=ot_sb[7][:, MB // 2:MB],
                ).then_inc(st_h, 16)
                for j in range(1, 8, 2):
                    scalar.wait_ge(st_sems[j], 32 if j != 7 else 16)
                scalar.wait_ge(st_h, 32)

            @block.gpsimd
            def _(gpsimd):
                gpsimd.memset(dummy_sb[:], 0.0).then_inc(dm, 1)
                gpsimd.dma_start(out=bias_sb[:], in_=bias[:]).then_inc(ld_b, 16)

            @block.tensor
            def _(tensor):
                # ~0.9 us of tiny matmuls on zeroed SBUF to get the HAM
                # activity window counting before real data lands. Bank 0
                # is overwritten by the first real start=True matmul.
                tensor.wait_ge(dm, 1)
                for _ in range(8):
                    tensor.matmul(
                        ps[0][:, 0:P], dummy_sb[:, 0:P], dummy_sb[:, 0:P],
                        start=True, stop=True,
                    )
                # Phase A (mb=0): k-outer over all 8 banks - each k-slice
                # feeds 8 matmuls the moment it lands.
                for k in range(KT):
                    if k == 0:
                        tensor.wait_ge(ld_w0[0], 16)
                        tensor.wait_ge(ld_x0[0], 16)
                    else:
                        tensor.wait_ge(ld_w[k - 1], 16)
                        tensor.wait_ge(ld_x[k - 1], 16)
                    for nt in range(NT):
                        if k == 0 and nt == 4:
                            tensor.wait_ge(ld_w0[1], 16)
                        inst = tensor.matmul(
                            ps[nt][:, :],
                            wt_sb[k][:, nt * P:(nt + 1) * P],
                            xt_sb[k][:, 0:MB],
                            start=(k == 0),
                            stop=(k == KT - 1),
                        )
                        if k == KT - 1:
                            inst.then_inc(mm, 1)
                # Phase B (mb=1): k-inner per group - completions land
                # ~1.8 us apart so evictions + stores pipeline.
                tensor.wait_ge(ld_x0[1], 16)
                for nt in range(NT):
                    tensor.wait_ge(ev, nt + 1)  # bank nt evicted (A)
                    inst = None
                    for k in range(KT):
                        inst = tensor.matmul(
                            ps[nt][:, :],
                            wt_sb[k][:, nt * P:(nt + 1) * P],
                            xt_sb[k][:, MB:2 * MB],
                            start=(k == 0),
                            stop=(k == KT - 1),
                        )
                    inst.then_inc(mm, 1)

            @block.vector
            def _(vector):
                vector.wait_ge(ld_b, 16)
                for g in range(NGROUPS - 1):
                    mb, nt = divmod(g, NT)
                    vector.wait_ge(mm, g + 1)
                    if g >= 8:
                        # ot slot g-8 reused: its store must be done
                        vector.wait_ge(st_sems[g - 8], 16)
                    vector.tensor_scalar_add(
                        ot_sb[g % 8][:],
                        ps[g % 8][:, :],
                        bias_sb[:, nt:nt + 1],
                    ).then_inc(ev, 1)
                # Last group in halves: first half's store overlaps the
                # second half's eviction, shortening the critical tail.
                vector.wait_ge(mm, NGROUPS)
                vector.wait_ge(st_sems[7], 16)
                for h in range(2):
                    vector.tensor_scalar_add(
                        ot_sb[7][:, h * (MB // 2):(h + 1) * (MB // 2)],
                        ps[7][:, h * (MB // 2):(h + 1) * (MB // 2)],
                        bias_sb[:, 7:8],
                    ).then_inc(ev_h, 1)

    return nc


_PROGRAM = None


def _get_program() -> bass.Bass:
    global _PROGRAM
    if _PROGRAM is None:
        _PROGRAM = build_program()
    return _PROGRAM


def make_in_maps(x: np.ndarray, W: np.ndarray, b: np.ndarray) -> list[dict]:
    WT = np.ascontiguousarray(W.T.astype(np.float16))
    bias = np.ascontiguousarray(
        b.astype(np.float32, copy=False).reshape(NT, P).T
    )
    in_maps = []
    for c in range(N_CORES):
        xT = np.ascontiguousarray(x[c * M:(c + 1) * M, :].T.astype(np.float16))
        in_maps.append({"xT": xT, "wT": WT, "bias": bias})
    return in_maps


def assemble_output(results: list[dict]) -> np.ndarray:
    out = np.empty((B, OUT_F), dtype=np.float32)
    for c in range(N_CORES):
        out[c * M:(c + 1) * M, :] = results[c]["outT"].T.astype(np.float32)
    return out


def kernel(x: np.ndarray, W: np.ndarray, b: np.ndarray) -> np.ndarray:
    nc = _get_program()
    in_maps = make_in_maps(np.asarray(x), np.asarray(W), np.asarray(b))
    res = run_bass_kernel_spmd(nc, in_maps, list(range(N_CORES)))
    return assemble_output(res.results)
